# revision 21
# baseline (speedup 1.0000x reference)
"""Multi-head forgetting attention on 8 trn2 cores.

Sharding: 4 heads per core as 2 head-pairs (hp), 1 batch per 4 cores.
Each core gets host-pre-transposed activations, its column slice of
Wq/Wk/Wv, row slice of Wo^T, and produces a (S, D) bf16 partial output
(hp-combined in PSUM); the host sums the 4 partials per batch (+ bo).

Engine assignment per score block (k-major [128k, span_q], h batched):
  PE:   scoresT = K^T.Q (bf16) with -100*(1-mask) accumulated via
        identity-matmul on mixed 128-blocks (masking free downstream)
  ACT:  tanh(0.5*gq[q]+0.5*gk[k]+0.5*gb) per (i,h), bias = per-part gk
  DVE:  one affine_mul_reduce per (aq,i): (0.5*t+0.5)*scores -> g3 f32
  ACT:  one exp per i-pair -> probs bf16
  PE:   attn@V with ones-column rowsum; PE-transpose; hp-combined O-proj
  DVE:  recip of rowsums [128,2] per q-slice; ott copies
  Pool: all PSUM->SBUF copies (Q/K/V/gq/gqb/out), normalize scaling
DMA queues (a DMA blocks its issuing sequencer for the whole transfer):
  sync: xq+xk streams; vector: xv stream + small consts;
  scalar: weights+mask preload only; gpsimd(SWDGE): output stores.
"""

import os
import sys

sys.path.insert(0, "/opt/trn_rl_repo")

import numpy as np
import ml_dtypes

bf16 = ml_dtypes.bfloat16

B, S, D, H = 2, 2048, 1024, 16
DK = 64
NCORES = 8
HPC = 2          # head-pairs per core (each pair = 2 heads = 128 cols)
CW = HPC * DK    # 128 per-head-pair width
P = 128
QTW = 512        # q tile width
NQT = S // QTW   # 4
NKT = S // P     # 16 k tiles
NSL = S // P     # 16 q slices
ND = D // P      # 8 contraction tiles

_CACHE = {}


def _prep_mask(mask):
    """Batch-union block table: 0 skip / 1 full / 2 mixed, plus per-batch
    -100*(1-mask) bf16 tiles ([k,q] orientation, stored [P, n, P]) for
    mixed blocks; the last tile is all -100 (for skip segments)."""
    m = np.asarray(mask).astype(bool)
    st = np.zeros((NKT, NSL), dtype=np.int8)
    mneg = [[] for _ in range(B)]
    midx = {}
    for i in range(NKT):
        for s in range(NSL):
            blks = [m[b, s * P:(s + 1) * P, i * P:(i + 1) * P]
                    for b in range(B)]
            alls = [blk.all() for blk in blks]
            anys = [blk.any() for blk in blks]
            if all(alls):
                st[i, s] = 1
            elif not any(anys):
                st[i, s] = 0
            else:
                st[i, s] = 2
                midx[(i, s)] = len(mneg[0])
                for b in range(B):
                    t = (-100.0 * (1.0 - blks[b].T)).astype(bf16)
                    mneg[b].append(np.ascontiguousarray(t))
    for b in range(B):
        mneg[b].append(np.full((P, P), -100.0, dtype=bf16))
    stacked = [np.ascontiguousarray(np.stack(x).transpose(1, 0, 2))
               for x in mneg]
    return st, stacked, midx


def _build(meta):
    import concourse.mybir as mybir
    import concourse.tile as tile
    from concourse import bacc

    st = meta["st"]
    midx = meta["midx"]
    n_m = meta["n_m"]
    gb = meta["gb"]
    use_bq = meta["use_bq"]
    use_bk = meta["use_bk"]
    use_bv = meta["use_bv"]

    f32 = mybir.dt.float32
    b16 = mybir.dt.bfloat16
    Act = mybir.ActivationFunctionType
    Alu = mybir.AluOpType

    # block tables (batch-union)
    iv_qt = {qt: [i for i in range(NKT)
                  if any(st[i, 4 * qt + j] for j in range(4))]
             for qt in range(NQT)}
    valid_i = {s: [i for i in range(NKT) if st[i, s]]
               for s in range(NSL)}
    mc = {qt: max((i // (QTW // P) for i in iv_qt[qt]), default=0)
          for qt in range(NQT)}

    nc = bacc.Bacc("TRN2", debug=False, enable_asserts=False,
                   num_devices=NCORES)

    xqt = nc.dram_tensor("xqt", (P, ND, S), b16, kind="ExternalInput")
    xkt = nc.dram_tensor("xkt", (P, ND, S), b16, kind="ExternalInput")
    xvt = nc.dram_tensor("xvt", (P, ND, S), b16, kind="ExternalInput")
    wqt = nc.dram_tensor("wqt", (P, HPC, D), b16, kind="ExternalInput")
    wkt = nc.dram_tensor("wkt", (P, HPC, D), b16, kind="ExternalInput")
    wvt = nc.dram_tensor("wvt", (P, HPC, D), b16, kind="ExternalInput")
    wot = nc.dram_tensor("wot", (P, HPC, D), b16, kind="ExternalInput")
    gqbd = nc.dram_tensor("gqbd", (P, HPC, HPC, S), b16,
                          kind="ExternalInput")
    gk05d = nc.dram_tensor("gk05d", (P, HPC, HPC, NKT), f32,
                           kind="ExternalInput")
    identd = nc.dram_tensor("identd", (P, P), b16, kind="ExternalInput")
    mnegd = nc.dram_tensor("mnegd", (P, n_m, P), b16,
                           kind="ExternalInput")
    bqt = nc.dram_tensor("bqt", (P, HPC, 1), f32, kind="ExternalInput")
    bkt = nc.dram_tensor("bkt", (P, HPC, 1), f32, kind="ExternalInput")
    bvt = nc.dram_tensor("bvt", (P, HPC, P), f32, kind="ExternalInput")
    outp = nc.dram_tensor("outp", (S, D), b16, kind="ExternalOutput")

    with tile.TileContext(nc) as tc:
        from contextlib import ExitStack
        with ExitStack() as ctx:
            cst = ctx.enter_context(tc.tile_pool(name="cst", bufs=1))
            per = ctx.enter_context(tc.tile_pool(name="per", bufs=1))
            strm = ctx.enter_context(tc.tile_pool(name="strm", bufs=4))
            work = ctx.enter_context(tc.tile_pool(name="work", bufs=2))
            prb = ctx.enter_context(tc.tile_pool(name="prb", bufs=1))
            ps = ctx.enter_context(
                tc.tile_pool(name="ps", bufs=2, space="PSUM"))

            # ---- constants ----
            wq_sb = cst.tile([P, HPC, D], b16, name="wq_sb")
            wk_sb = cst.tile([P, HPC, D], b16, name="wk_sb")
            wv_sb = cst.tile([P, HPC, D], b16, name="wv_sb")
            wo_sb = cst.tile([P, HPC, D], b16, name="wo_sb")
            id_sb = cst.tile([P, P], b16, name="id_sb")
            mneg_sb = cst.tile([P, n_m, P], b16, name="mneg_sb")
            bq_sb = cst.tile([P, HPC, 1], f32, name="bq_sb")
            bk_sb = cst.tile([P, HPC, 1], f32, name="bk_sb")
            bv_sb = cst.tile([P, HPC, P], f32, name="bv_sb")
            gk_sb = [[per.tile([P, NKT], f32, name=f"gk{hp}{h}_sb",
                               tag=f"gk{hp}{h}") for h in range(HPC)]
                     for hp in range(HPC)]
            gqb_sb = [[per.tile([P, S], b16, name=f"gqb{hp}{h}_sb",
                                tag=f"gqb{hp}{h}") for h in range(HPC)]
                      for hp in range(HPC)]
            # startup: gate tensors + proj weights first (tanh-critical),
            # V/O weights and mask tiles on the idle gpsimd queue
            for hp in range(HPC):
                for h in range(HPC):
                    nc.scalar.dma_start(gqb_sb[hp][h][:],
                                        gqbd[:, hp, h, :])
                    nc.scalar.dma_start(gk_sb[hp][h][:],
                                        gk05d[:, hp, h, :])
            nc.scalar.dma_start(id_sb[:], identd[:, :])
            nc.scalar.dma_start(wq_sb[:], wqt[:, :, :])
            nc.scalar.dma_start(wk_sb[:], wkt[:, :, :])
            nc.gpsimd.dma_start(wv_sb[:], wvt[:, :, :])
            nc.gpsimd.dma_start(wo_sb[:], wot[:, :, :])
            nc.gpsimd.dma_start(mneg_sb[:], mnegd[:, :, :])
            if use_bq:
                nc.scalar.dma_start(bq_sb[:], bqt[:, :, :])
            if use_bk:
                nc.scalar.dma_start(bk_sb[:], bkt[:, :, :])
            if use_bv:
                nc.scalar.dma_start(bv_sb[:], bvt[:, :, :])

            # per-chunk activation tiles (separate tiles avoid false
            # write-after-read deps between proj(c+1) and attn(c))
            qt_c = [[per.tile([P, QTW], b16, name=f"q{hp}c{c}",
                              tag=f"q{hp}c{c}") for c in range(NQT)]
                    for hp in range(HPC)]
            kt_c = [[per.tile([P, QTW], b16, name=f"k{hp}c{c}",
                              tag=f"k{hp}c{c}") for c in range(NQT)]
                    for hp in range(HPC)]
            NVC = (QTW // P) * (DK + 1)
            v_c = [[[per.tile([P, NVC], b16, name=f"v{hp}{h}c{c}",
                              tag=f"v{hp}{h}c{c}") for c in range(NQT)]
                    for h in range(HPC)] for hp in range(HPC)]
            for hp in range(HPC):
                for h in range(HPC):
                    for c in range(NQT):
                        nc.vector.memset(
                            v_c[hp][h][c][:, DK::DK + 1], 1.0)

            def psum_big(name):
                return ps.tile([P, HPC, QTW], f32, tag="big", name=name,
                               bufs=2)

            def psum_att(name):
                return ps.tile([P, HPC, 66], f32, tag="att", name=name,
                               bufs=2)

            def psum_mis(name):
                return ps.tile([P, QTW], f32, tag="mis", name=name,
                               bufs=2)

            def emit_proj(qt):
                q0 = qt * QTW
                for (xsrc, wsb, osb, bias_sb, use_b, xtag) in (
                        (xqt, wq_sb, qt_c, bq_sb, use_bq, "xq"),
                        (xkt, wk_sb, kt_c, bk_sb, use_bk, "xk")):
                    pps = psum_big(f"pp_{xtag}")
                    xt = strm.tile([P, ND, QTW], b16, tag="x", bufs=3,
                                   name=xtag)
                    nc.sync.dma_start(xt[:, :ND // 2, :],
                                      xsrc[:, :ND // 2, q0:q0 + QTW])
                    nc.sync.dma_start(xt[:, ND // 2:, :],
                                      xsrc[:, ND // 2:, q0:q0 + QTW])
                    for dt in range(ND):
                        for hp in range(HPC):
                            nc.tensor.matmul(
                                pps[:, hp, :],
                                lhsT=wsb[:, hp, dt * P:(dt + 1) * P],
                                rhs=xt[:, dt, :],
                                start=(dt == 0), stop=(dt == ND - 1))
                    for hp in range(HPC):
                        dst = osb[hp][qt][:]
                        if use_b:
                            nc.scalar.activation(
                                dst, pps[:, hp, :], Act.Identity,
                                bias=bias_sb[:, hp, :])
                        else:
                            nc.vector.tensor_copy(dst, pps[:, hp, :])

                xv = strm.tile([P, ND, QTW], b16, name="xv", tag="xv",
                               bufs=2)
                nc.gpsimd.dma_start(xv[:, :ND // 2, :],
                                    xvt[:, :ND // 2, q0:q0 + QTW])
                nc.gpsimd.dma_start(xv[:, ND // 2:, :],
                                    xvt[:, ND // 2:, q0:q0 + QTW])
                for hp in range(HPC):
                    for sj in range(QTW // P):
                        vps = psum_att("vps")
                        for h in range(HPC):
                            for dt in range(ND):
                                nc.tensor.matmul(
                                    vps[:, h, :DK],
                                    lhsT=xv[:, dt, sj * P:(sj + 1) * P],
                                    rhs=wv_sb[:, hp,
                                              dt * P + h * DK:
                                              dt * P + (h + 1) * DK],
                                    start=(dt == 0), stop=(dt == ND - 1),
                                    skip_group_check=True)
                        for h in range(HPC):
                            vv = vps[:, h, :DK]
                            if use_bv:
                                nc.vector.tensor_add(
                                    vv, vv,
                                    bv_sb[:, hp, h * DK:(h + 1) * DK])
                            nc.vector.tensor_copy(
                                v_c[hp][h][qt][:, sj * (DK + 1):
                                               sj * (DK + 1) + DK], vv)

            projected = 0

            def ensure_proj(c):
                nonlocal projected
                while projected <= c:
                    emit_proj(projected)
                    projected += 1

            for aq in range(NQT):
                ensure_proj(mc[aq])
                a0 = aq * QTW
                otts = []
                for hp in range(HPC):
                    probs = {}
                    iv = iv_qt[aq]
                    # group adjacent full-span i's in pairs (one exp)
                    groups = []
                    for i in iv:
                        sjlo = min(j for j in range(QTW // P)
                                   if st[i, aq * (QTW // P) + j])
                        if (sjlo == 0 and groups
                                and len(groups[-1]) == 1
                                and groups[-1][0][1] == 0):
                            groups[-1].append((i, 0))
                        else:
                            groups.append([(i, sjlo)])
                    npair = 0
                    nsing = 0
                    for grp in groups:
                        ng = len(grp)
                        g3 = work.tile([P, 2, HPC, QTW], f32,
                                       tag="g3", name="g3")
                        if ng == 2:
                            p3 = prb.tile([P, 2, HPC, QTW], b16,
                                          tag=f"pp{npair}",
                                          name=f"pp{npair}", bufs=2)
                            npair += 1
                        else:
                            p3 = prb.tile([P, 1, HPC, QTW], b16,
                                          tag=f"psg{nsing}",
                                          name=f"psg{nsing}", bufs=2)
                            nsing += 1
                        goff = QTW
                        for gi, (i, sjlo) in enumerate(grp):
                            off = sjlo * P
                            goff = min(goff, off)
                            sp3 = psum_big(f"sc{i}")
                            tnh = work.tile([P, HPC, QTW], b16,
                                            tag="tnh", name="tnh",
                                            bufs=4)
                            for h in range(HPC):
                                hsl = slice(h * DK, (h + 1) * DK)
                                # scores with mask folded in via PE
                                segs = []
                                for sj in range(sjlo, QTW // P):
                                    sv = st[i, aq * (QTW // P) + sj]
                                    if sv == 2:
                                        segs.append(["m", sj, sj + 1])
                                    elif sv == 0:
                                        segs.append(["z", sj, sj + 1])
                                    elif segs and segs[-1][0] == "c":
                                        segs[-1][2] = sj + 1
                                    else:
                                        segs.append(["c", sj, sj + 1])
                                for kind, j0, j1 in segs:
                                    c0, c1 = j0 * P, j1 * P
                                    dst = sp3[:, h, c0:c1]
                                    if kind == "z":
                                        nc.tensor.matmul(
                                            dst, lhsT=id_sb[:],
                                            rhs=mneg_sb[:, n_m - 1, :],
                                            start=True, stop=True,
                                            skip_group_check=True)
                                        continue
                                    nc.tensor.matmul(
                                        dst,
                                        lhsT=kt_c[hp][i // 4][
                                            hsl, (i % 4) * P:
                                                 (i % 4 + 1) * P],
                                        rhs=qt_c[hp][aq][hsl, c0:c1],
                                        start=True, stop=(kind == "c"),
                                        skip_group_check=True)
                                    if kind == "m":
                                        mi = midx[(i, aq * (QTW // P)
                                                   + j0)]
                                        nc.tensor.matmul(
                                            dst, lhsT=id_sb[:],
                                            rhs=mneg_sb[:, mi, :],
                                            start=False, stop=True,
                                            skip_group_check=True)
                                nc.scalar.activation(
                                    tnh[:, h, off:],
                                    gqb_sb[hp][h][:, a0 + off:a0 + QTW],
                                    Act.Tanh,
                                    bias=gk_sb[hp][h][:, i:i + 1],
                                    scale=0.5)
                            acc = work.tile([P, 1], f32, tag="acc",
                                            name="acc", bufs=6)
                            nc.vector.affine_mul_reduce(
                                g3[:, gi, :, off:], acc[:],
                                tnh[:, :, off:],
                                sp3[:, :, off:], 0.5, 0.5)
                            probs[i] = (p3, gi)
                        nc.scalar.activation(
                            p3[:, :, :, goff:], g3[:, :ng, :, goff:],
                            Act.Exp)

                    if hp == 0 and projected < NQT:
                        emit_proj(projected)
                        projected += 1

                    # attn @ V, normalize, transpose per 128-q-slice
                    ott = work.tile([P, QTW], b16, tag=f"ott{hp}",
                                    name=f"ott{hp}")
                    otts.append(ott)
                    for sj in range(QTW // P):
                        s = aq * (QTW // P) + sj
                        ops = psum_att("ops")
                        vi = valid_i[s]
                        for h in range(HPC):
                            if not vi:
                                nc.vector.memset(ops[:, h, :DK], 0.0)
                                nc.vector.memset(
                                    ops[:, h, DK:DK + 1], 1.0)
                            for n, i in enumerate(vi):
                                pt, gi = probs[i]
                                nc.tensor.matmul(
                                    ops[:, h, :DK + 1],
                                    lhsT=pt[:, gi, h,
                                            sj * P:(sj + 1) * P],
                                    rhs=v_c[hp][h][i // 4][
                                        :, (i % 4) * (DK + 1):
                                           (i % 4 + 1) * (DK + 1)],
                                    start=(n == 0),
                                    stop=(n == len(vi) - 1),
                                    skip_group_check=True)
                        rec = work.tile([P, 2], f32, tag="rec",
                                        name="rec", bufs=4)
                        nc.vector.reciprocal_approx_fast(
                            rec[:], ops[:, :, DK])
                        onat = work.tile([P, P], b16, tag="onat",
                                         name="onat")
                        for h in range(HPC):
                            nc.vector.tensor_scalar_mul(
                                onat[:, h * DK:(h + 1) * DK],
                                ops[:, h, :DK], rec[:, h:h + 1])
                        trp = psum_mis("trp")
                        trpv = trp[:, :64].bitcast(b16)
                        nc.tensor.transpose(trpv, onat[:], id_sb[:])
                        nc.vector.tensor_copy(
                            ott[:, sj * P:(sj + 1) * P], trpv)

                # hp-combined O-projection per 128-q-slice
                for sj in range(QTW // P):
                    s = aq * (QTW // P) + sj
                    po = work.tile([P, 2 * QTW], b16, tag="po",
                                   name="po", bufs=3)
                    for nt in range(2):
                        pps2 = psum_mis("fps")
                        for hp in range(HPC):
                            nc.tensor.matmul(
                                pps2[:],
                                lhsT=otts[hp][:, sj * P:(sj + 1) * P],
                                rhs=wo_sb[:, hp,
                                          nt * QTW:(nt + 1) * QTW],
                                start=(hp == 0), stop=(hp == 1))
                        nc.vector.tensor_copy(
                            po[:, nt * QTW:(nt + 1) * QTW], pps2[:])
                    nc.gpsimd.dma_start(
                        outp[s * P:(s + 1) * P, :], po[:])
    nc.compile()
    return nc


def _host_prep(inputs):
    q = np.asarray(inputs["query"], np.float32)
    k = np.asarray(inputs["key"], np.float32)
    v = np.asarray(inputs["value"], np.float32)
    mask = np.asarray(inputs["mask"])
    Wq = np.asarray(inputs["Wq"], np.float32)
    Wk = np.asarray(inputs["Wk"], np.float32)
    Wv = np.asarray(inputs["Wv"], np.float32)
    Wo = np.asarray(inputs["Wo"], np.float32)
    bq = np.asarray(inputs["bq"], np.float32)
    bk = np.asarray(inputs["bk"], np.float32)
    bv = np.asarray(inputs["bv"], np.float32)
    bo = np.asarray(inputs["bo"], np.float32)
    wgq = np.asarray(inputs["wgq"], np.float32)
    wgk = np.asarray(inputs["wgk"], np.float32)
    gb = float(np.asarray(inputs["gb"]))

    st, mneg_b, midx = _prep_mask(mask)

    xt_b = [[np.ascontiguousarray(
                x[b].T.reshape(ND, P, S).transpose(1, 0, 2)).astype(bf16)
             for b in range(B)]
            for x in (q, k, v)]

    def wslice(W, cols, scale=1.0):
        wt = (W.T[:, cols:cols + CW] * scale).astype(bf16)
        return np.ascontiguousarray(
            wt.reshape(ND, P, CW).transpose(1, 0, 2).reshape(P, D))

    scale = 1.0 / np.sqrt(DK)
    ident = np.eye(P, dtype=bf16)

    meta = {
        "st": st, "midx": midx, "n_m": mneg_b[0].shape[1], "gb": gb,
        "use_bq": bool(np.any(bq)), "use_bk": bool(np.any(bk)),
        "use_bv": bool(np.any(bv)),
    }

    # host-side gate vectors per (batch, head): tiny matvecs
    # gq_bh[b, hd, q] = (query[b] @ Wq.T + bq)[:, hd-cols] @ wgq
    wq_g = (Wq.T.reshape(D, H, DK) @ wgq)      # [D, H]
    wk_g = (Wk.T.reshape(D, H, DK) @ wgk)      # [D, H]
    bq_g = bq.reshape(H, DK) @ wgq             # [H]
    bk_g = bk.reshape(H, DK) @ wgk             # [H]
    gq_bh = np.einsum('bsd,dh->bhs', q, wq_g) + bq_g[None, :, None]
    gk_bh = np.einsum('bsd,dh->bhs', k, wk_g) + bk_g[None, :, None]

    ngrp = NCORES // B          # head-groups per batch
    in_maps = []
    for c in range(NCORES):
        bc = c // ngrp          # batch of this core
        hg = c % ngrp           # head-group
        cols = [(hg * HPC + 0) * CW, (hg * HPC + 1) * CW]
        im = {
            "xqt": xt_b[0][bc], "xkt": xt_b[1][bc], "xvt": xt_b[2][bc],
            "wqt": np.ascontiguousarray(np.stack(
                [wslice(Wq, cl, scale) for cl in cols]).transpose(1, 0, 2)),
            "wkt": np.ascontiguousarray(np.stack(
                [wslice(Wk, cl) for cl in cols]).transpose(1, 0, 2)),
            "wvt": np.ascontiguousarray(np.stack(
                [wslice(Wv, cl) for cl in cols]).transpose(1, 0, 2)),
            "wot": np.ascontiguousarray(np.stack(
                [Wo.T[cl:cl + CW, :].astype(bf16)
                 for cl in cols]).transpose(1, 0, 2)),
            "gqbd": np.ascontiguousarray(np.broadcast_to(
                np.stack([np.stack([gq_bh[bc, (cols[hp] // DK) + h]
                                    for h in range(HPC)])
                          for hp in range(HPC)])[None],
                (P, HPC, HPC, S)).astype(bf16)),
            "gk05d": np.ascontiguousarray(np.stack(
                [np.stack(
                    [(0.5 * gk_bh[bc, (cols[hp] // DK) + h] + 0.5 * gb)
                     .reshape(NKT, P).T.astype(np.float32)
                     for h in range(HPC)])
                 for hp in range(HPC)]).transpose(2, 0, 1, 3)),
            "identd": ident, "mnegd": mneg_b[bc],
            "bqt": np.ascontiguousarray(np.stack(
                [(bq[cl:cl + CW] * scale).reshape(P, 1).astype(np.float32)
                 for cl in cols]).transpose(1, 0, 2)),
            "bkt": np.ascontiguousarray(np.stack(
                [bk[cl:cl + CW].reshape(P, 1).astype(np.float32)
                 for cl in cols]).transpose(1, 0, 2)),
            "bvt": np.ascontiguousarray(np.stack(
                [np.tile(bv[cl:cl + CW], (P, 1)).astype(np.float32)
                 for cl in cols]).transpose(1, 0, 2)),
        }
        in_maps.append(im)
    return meta, in_maps, bo


def kernel(**inputs):
    meta, in_maps, bo = _host_prep(inputs)

    key = (meta["st"].tobytes(), meta["gb"], meta["use_bq"],
           meta["use_bk"], meta["use_bv"], meta["n_m"])
    if key not in _CACHE:
        _CACHE[key] = _build(meta)
    nc = _CACHE[key]

    from concourse.bass_utils import run_bass_kernel_spmd
    res = run_bass_kernel_spmd(
        nc, in_maps, core_ids=list(range(NCORES)),
        trace=bool(int(os.environ.get("KERNEL_TRACE", "0"))))
    out = np.zeros((B, S, D), np.float32)
    ngrp = NCORES // B
    for c, r in enumerate(res.results):
        out[c // ngrp] += r["outp"].astype(np.float32)
    out += bo
    if res.exec_time_ns is not None:
        print(f"HW exec time: {res.exec_time_ns} ns")
    return out
